# revision 8
# baseline (speedup 1.0000x reference)
"""Causal single-head attention on 8 Trainium2 NeuronCores.

Problem: x[B=4,T=4096,C=512] fp32, Wk/Wq/Wv[C,H=64] -> out[B,T,H].

Sharding: 2 cores per batch element. Within a pair, the KEY tiles (128 keys
each, 32 tiles) are interleaved by parity: core parity p owns key tiles
{p, p+2, p+4, ...}. Each core computes, for ALL queries of its batch, the
unnormalized partial softmax numerator (sum_k exp(s) * v) and denominator
(sum_k exp(s)) over its own keys only; the host sums the two partials and
divides. exp() without max-subtraction is safe here (scores ~ N(0,1)).

This makes every core's program byte-identical (SPMD requirement): the
causal structure is identical for both parities (query block i needs
exactly 2i+2 local key tiles on either parity), and all parity/batch
differences live in the DMA'd data:
  - xt: x[b].T in fp16, columns permuted to [own-parity key tiles | rest]
  - masks: additive causal masks for the two diagonal tiles of each block
  - query columns inside a 512-block are streamed in a fixed parity-dependent
    tile order; the host un-permutes the output columns.

Compute is fp16 (10 mantissa bits, ~4e-4 end-to-end rel err, full PE rate;
fp32/fp32r matmuls run 3-4x slower and keep the PE clock-gated cold).
PSUM accumulation is fp32 throughout.

On-device dataflow per core:
  K^T,V^T = [Wk|Wv]^T @ xt_kv   (one fused pass, M=128)
  Q^T     = Wq^T @ xt           (all queries)
  V^T -> V via PE transpose-mode
  per query block qb (512 queries), per local key tile PAIR lp in 0..qb:
     S^T[128k,2x512q] = (K^T tiles).T @ Q^T block   (two MMs, one PSUM tile)
     S += causal mask  (last pair only, one DVE add over [128,1024])
     P = exp(S*0.125)  (one ACT op [128,1024] -> SBUF fp16)
     O'[65,512q] += [V|1].T @ P halves              (PSUM accumulate)
  O' -> SBUF -> DRAM partial [65, 4096]
"""

import os
import numpy as np

B, T, C, H = 4, 4096, 512, 64
NKT = T // 128          # 32 natural key tiles per batch
NLOC = NKT // 2         # 16 local key tiles per core
QB = T // 512           # 8 query blocks
SCALE = float(H) ** -0.5

_CACHE = {}
LAST_RESULTS = None


def _build_program():
    from contextlib import ExitStack
    import concourse.tile as tile
    from concourse import bacc, mybir
    from concourse.masks import make_identity

    F32 = mybir.dt.float32
    F16 = mybir.dt.float16
    EXP = mybir.ActivationFunctionType.Exp

    nc = bacc.Bacc("TRN2", target_bir_lowering=False, debug=False,
                   num_devices=8)

    xt = nc.dram_tensor("xt", (C, T), F16, kind="ExternalInput").ap()
    wkk = nc.dram_tensor("wkk", (128, 512), F16, kind="ExternalInput").ap()
    wqq = nc.dram_tensor("wqq", (128, 512), F16, kind="ExternalInput").ap()
    wv = nc.dram_tensor("wv", (128, 4 * H), F16, kind="ExternalInput").ap()
    masks = nc.dram_tensor("masks", (128, 1024), F32, kind="ExternalInput").ap()
    opart = nc.dram_tensor("opart", (65, T), F32, kind="ExternalOutput").ap()

    with tile.TileContext(nc) as tc, ExitStack() as ctx:
        persist = ctx.enter_context(tc.tile_pool(name="persist", bufs=1))
        xtp = ctx.enter_context(tc.tile_pool(name="xtp", bufs=4))
        vst_p = ctx.enter_context(tc.tile_pool(name="vst", bufs=2))
        p_pool = ctx.enter_context(tc.tile_pool(name="pp", bufs=3))
        out_p = ctx.enter_context(tc.tile_pool(name="outp", bufs=2))
        ps_big = ctx.enter_context(tc.tile_pool(name="psb", bufs=3, space="PSUM"))
        ps_sm = ctx.enter_context(tc.tile_pool(name="pss", bufs=2, space="PSUM"))

        # ---- persistent SBUF ----
        wkk_sb = persist.tile([128, 4 * 128], F16)
        wqq_sb = persist.tile([128, 4 * 128], F16)
        wv_sb = persist.tile([128, 4 * H], F16)
        mask_sb = persist.tile([128, 1024], F32)
        kTq_sb = persist.tile([128, NLOC * 128], F16)
        qTq_sb = persist.tile([128, T], F16)
        v_sb = persist.tile([128, NLOC * 65], F16)
        ident = persist.tile([64, 64], F16)

        nc.sync.dma_start(wkk_sb[:], wkk[:])
        make_identity(nc, ident[:])
        # ones column of [V|1]: out = in*0 + 1
        v_ones = v_sb[:].rearrange("p (l e) -> p l e", e=65)[:, :, 64:65]
        nc.scalar.activation(v_ones, mask_sb[:, 0:NLOC],
                             mybir.ActivationFunctionType.Copy,
                             bias=1.0, scale=0.0)

        # split loads across both HWDGE rings (sync + scalar) to double
        # effective DMA issue throughput; kv halves first
        nc.scalar.dma_start(wqq_sb[:], wqq[:])
        xt_sb = []
        for cc in range(4):
            t = xtp.tile([128, T], F16, tag="xt")
            eng = nc.sync if cc % 2 == 0 else nc.scalar
            eng.dma_start(t[:, 0:T // 2],
                          xt[128 * cc:128 * (cc + 1), 0:T // 2])
            xt_sb.append(t)
        for cc in range(4):
            eng = nc.sync if cc % 2 == 0 else nc.scalar
            eng.dma_start(xt_sb[cc][:, T // 2:T],
                          xt[128 * cc:128 * (cc + 1), T // 2:T])
        nc.sync.dma_start(wv_sb[:], wv[:])
        nc.scalar.dma_start(mask_sb[:], masks[:])

        # PE warmup: ~60 dummy matmuls on scratch during the DMA preamble so
        # the HAM clock-gate is released (K=8/8) before real compute starts
        warm_sc = persist.tile([64, 64], F16)
        nc.gpsimd.memset(warm_sc[:], 0.0)
        pwarm = ps_sm.tile([64, 64], F32, tag="sm")
        for _w in range(80):
            nc.tensor.matmul(pwarm[:], warm_sc[:], warm_sc[:],
                             start=True, stop=True, skip_group_check=True)

        qT_v = qTq_sb[:].rearrange("p (half l k) -> p half l k", half=2, k=128)

        def k_proj(tb):
            # [Wk|Wk]: K^T duplicated across both partition halves for the
            # row-packed S matmuls
            pkk = ps_big.tile([128, 1024], F32, tag="big")
            for cc in range(4):
                nc.tensor.matmul(
                    pkk[:, 0:512], wkk_sb[:, 128 * cc:128 * (cc + 1)],
                    xt_sb[cc][:, 512 * tb:512 * (tb + 1)],
                    start=(cc == 0), stop=(cc == 3))
            nc.vector.tensor_copy(kTq_sb[:, 512 * tb:512 * (tb + 1)],
                                  pkk[:, 0:512])

        def v_proj(tb):
            pvv = ps_sm.tile([64, 512], F32, tag="sm")
            for cc in range(4):
                nc.tensor.matmul(
                    pvv[:], wv_sb[:, H * cc:H * (cc + 1)],
                    xt_sb[cc][:, 512 * tb:512 * (tb + 1)],
                    start=(cc == 0), stop=(cc == 3))
            vt_st = vst_p.tile([64, 512], F16, tag="vst")
            nc.vector.tensor_copy(vt_st[:], pvv[:])
            for j in range(4):
                l = 4 * tb + j
                pv = ps_sm.tile([128, 64], F16, tag="sm")
                nc.tensor.transpose(pv[:], vt_st[:, 128 * j:128 * (j + 1)],
                                    ident[:])
                nc.vector.tensor_copy(v_sb[:, 65 * l:65 * l + 64], pv[:])

        def q_proj(pb):
            # [Wq|Wq]: Q^T duplicated across both partition halves
            pqq = ps_big.tile([128, 1024], F32, tag="big")
            for cc in range(4):
                nc.tensor.matmul(
                    pqq[:, 0:512], wqq_sb[:, 128 * cc:128 * (cc + 1)],
                    xt_sb[cc][:, 512 * pb:512 * (pb + 1)],
                    start=(cc == 0), stop=(cc == 3))
            nc.vector.tensor_copy(qTq_sb[:, 512 * pb:512 * (pb + 1)],
                                  pqq[:, 0:512])

        def attention(qb):
            npair = qb + 1
            q_lo = qT_v[0:64, :, 2 * qb:2 * qb + 2, :]
            q_hi = qT_v[64:128, :, 2 * qb:2 * qb + 2, :]
            po = ps_sm.tile([65, 512], F32, tag="sm")
            ps_t = {}
            p_t = {}
            for step in range(npair + 2):
                if step < npair:        # S pair, row-packed (K=64 each)
                    lp = step
                    ps = ps_big.tile([128, 1024], F32, tag="big")
                    l0, l1 = 2 * lp, 2 * lp + 1
                    nc.tensor.matmul(ps[:, 0:512],
                                     kTq_sb[0:64, 128 * l0:128 * (l0 + 1)],
                                     q_lo, start=True, stop=True,
                                     tile_position=(0, 0))
                    nc.tensor.matmul(ps[:, 512:1024],
                                     kTq_sb[64:128, 128 * l1:128 * (l1 + 1)],
                                     q_hi, start=True, stop=True,
                                     tile_position=(64, 0))
                    ps_t[lp] = ps
                if 1 <= step <= npair:  # mask + exp
                    le = step - 1
                    ps = ps_t.pop(le)
                    if le == npair - 1:
                        nc.vector.tensor_add(ps[:], ps[:], mask_sb[:])
                    p_sb = p_pool.tile([128, 1024], F16, tag="p")
                    nc.scalar.activation(p_sb[:], ps[:], EXP, scale=SCALE)
                    p_t[le] = p_sb
                if step >= 2:           # AV accumulate
                    la = step - 2
                    p_sb = p_t.pop(la)
                    for h in range(2):
                        l = 2 * la + h
                        nc.tensor.matmul(po[:], v_sb[:, 65 * l:65 * (l + 1)],
                                         p_sb[:, 512 * h:512 * (h + 1)],
                                         start=(l == 0), stop=(l == 2 * qb + 1))
            o_sb = out_p.tile([65, 512], F32, tag="o")
            nc.vector.tensor_copy(o_sb[:], po[:])
            nc.sync.dma_start(opart[:, 512 * qb:512 * (qb + 1)], o_sb[:])

        # interleave phases: attention on block qb only needs K/V tiles from
        # kv_proj(<= qb//2) and Q columns from q_proj(qb//2, 4+qb//2)
        for tb in range(4):
            k_proj(tb)
            v_proj(tb)
            q_proj(tb)
            q_proj(4 + tb)
            attention(2 * tb)
            attention(2 * tb + 1)

    nc.compile()
    return nc


def _prep_inputs(x, Wk, Wq, Wv):
    """Per-core input marshalling (layout + fp16 cast, no math)."""
    def swz(w):
        # [C, m] -> [128, 4*m]: chunk cc (rows 128cc..) at free cols m*cc..
        m = w.shape[1]
        return np.ascontiguousarray(
            w.reshape(4, 128, m).transpose(1, 0, 2).reshape(128, 4 * m)
        ).astype(np.float16)

    wkk = swz(np.concatenate([Wk, Wk], axis=1))
    wqq = swz(np.concatenate([Wq, Wq], axis=1))
    wv = swz(Wv)
    mask_cache = {}
    in_maps = []
    for core in range(8):
        b, par = core // 2, core % 2
        xT = np.ascontiguousarray(x[b].T).astype(np.float16)   # [C, T]
        tiles = xT.reshape(C, NKT, 128)
        kv = tiles[:, par::2, :].reshape(C, NLOC * 128)
        oth = tiles[:, 1 - par::2, :].reshape(C, NLOC * 128)
        xt_perm = np.ascontiguousarray(np.concatenate([kv, oth], axis=1))

        if par not in mask_cache:
            J = [par, 2 + par, 1 - par, 3 - par]
            m = np.full((128, 1024), -1e9, np.float32)
            ks = np.arange(128)[:, None]
            qr = np.arange(128)[None, :]
            for mi, off in enumerate((par, 2 + par)):
                for s in range(4):
                    cond = (128 * off + ks) <= (128 * J[s] + qr)
                    m[:, 512 * mi + 128 * s:512 * mi + 128 * (s + 1)] = \
                        np.where(cond, 0.0, -1e9)
            mask_cache[par] = m
        in_maps.append({"xt": xt_perm, "wkk": wkk, "wqq": wqq, "wv": wv,
                        "masks": mask_cache[par]})
    return in_maps


def _combine(results):
    """Un-permute query columns, sum partials across the core pairs, divide."""
    out = np.empty((B, T, H), np.float32)
    for b in range(4):
        nats = []
        for par in range(2):
            J = [par, 2 + par, 1 - par, 3 - par]
            r = results[2 * b + par]["opart"].reshape(65, QB, 4, 128)
            nat = np.empty_like(r)
            for s in range(4):
                nat[:, :, J[s], :] = r[:, :, s, :]
            nats.append(nat.reshape(65, T))
        num = nats[0][:64] + nats[1][:64]
        den = nats[0][64] + nats[1][64]
        out[b] = (num / den[None, :]).T
    return out


def kernel(x, Wk, Wq, Wv):
    global LAST_RESULTS
    from concourse.bass_utils import run_bass_kernel_spmd

    if "nc" not in _CACHE:
        _CACHE["nc"] = _build_program()
    nc = _CACHE["nc"]

    in_maps = _prep_inputs(np.asarray(x, np.float32), np.asarray(Wk),
                           np.asarray(Wq), np.asarray(Wv))
    trace = bool(int(os.environ.get("ATTN_TRACE", "0")))
    res = run_bass_kernel_spmd(nc, in_maps, core_ids=list(range(8)),
                               trace=trace)
    LAST_RESULTS = res
    return _combine(res.results)


if __name__ == "__main__":
    rng = np.random.default_rng(0)
    x = rng.standard_normal((B, T, C), dtype=np.float32)
    Wk = (rng.standard_normal((C, H)) * C ** -0.5).astype(np.float32)
    Wq = (rng.standard_normal((C, H)) * C ** -0.5).astype(np.float32)
    Wv = (rng.standard_normal((C, H)) * C ** -0.5).astype(np.float32)
    out = kernel(x, Wk, Wq, Wv)
    k = x @ Wk; q = x @ Wq; v = x @ Wv
    s = np.einsum('bqh,bkh->bqk', q, k) * SCALE
    mask = np.tril(np.ones((T, T), dtype=bool))
    s = np.where(mask, s, -np.inf)
    p = np.exp(s - s.max(-1, keepdims=True))
    p /= p.sum(-1, keepdims=True)
    ref = np.einsum('bqk,bkh->bqh', p, v)
    err = np.abs(out - ref).max() / np.abs(ref).max()
    print("rel err vs numpy:", err)
